# revision 6
# baseline (speedup 1.0000x reference)
"""Cross-attention kernel for Trainium2 (8 NeuronCores, SPMD).

Problem: B=4, LQ=LK=4096, H=256
  query = q @ Wq.T + bq ; keys = k @ Wk.T + bk ; values = v @ Wv.T + bv
  out = softmax(query @ keys.T / sqrt(H)) @ values

Sharding: core i -> batch i//2, query rows (i%2)*2048 .. +2048.
K/V for the batch are replicated across the 2 cores sharing it.

Device algorithm (PE contracts over the partition dim):
  - scores are algebraically refactored:
      s[q,k] = (q M)_q k_k^T + t_q + u_k,  M = Wq.T @ Wk
      t_q -- constant per softmax row: cancels, dropped
      u_k = (k·(Wk.T bq) + bq·bk)/sqrt(H) -- per-key scalar; exp(u_k) is
            folded into the V rows on the host (V' = e^u [v, 1]), so the
            device exp needs NO bias and activations can span k-tiles.
  - qM and k are quantized to fp8e4 on the host (RNE); the scores matmul
    runs in DoubleRow perf mode: one matmul per k-tile contracts all 256
    h in a single pass ([128, 2, x] APs pair h and h+128 per partition),
    2x the bf16 rate.
  - scores are computed transposed ([k, q]) so exp(scores) = P^T is born
    k-major; P is materialized fp16 (attention here is diffuse, so fp8
    P/V error would not average down enough -- fp16 keeps P@V exact-ish
    while fp8 scores stay within the error budget).
  - softmax skips max-subtraction (scaled scores stay within ~+-4).
  - exp runs on ScalarE over [128, 1024] k-tile PAIRS (two PSUM banks per
    activation) to amortize the ~352-cycle fixed cost per ACTIVATE.
  - P@V uses P^T windows as stationary and V' ([k, 257] fp16, e^u row
    scale, ones-column) as moving: output column 256 is the softmax
    denominator and the context lands in natural [q, h] layout.
  - score and P@V matmuls are interleaved per k-tile-pair (P@V lags LAGP
    pairs) so the exp's ScalarE latency hides behind P@V work on PE; each
    chunk drains qw-major with the normalize fused per q-window.
"""

import os
import sys

import numpy as np

sys.path.insert(0, "/opt/trn_rl_repo")

import ml_dtypes

B, LQ, LK, H = 4, 4096, 4096, 256
P = 128
HO = H // P            # 2 h-tiles
NCORES = 8
NQ = LQ * B // NCORES  # 2048 q rows per core
QC = 512               # q chunk (scores tile width)
NQC = NQ // QC         # 4
QW = QC // P           # 4 q-windows per chunk
KT = LK // P           # 32 k tiles
KTP = KT // 2          # 16 k-tile pairs
HA = H + 1             # V augmented with ones column
LAGP = 3               # P@V lags scores by this many k-tile pairs
SCALE = 1.0 / np.sqrt(np.float32(H))  # 1/16

_F8 = ml_dtypes.float8_e4m3
_F16 = np.float16

_NC_CACHE = None


def _build_nc():
    """Build the single-core Bass program (same program runs SPMD on 8 cores)."""
    import concourse.bass as bass
    import concourse.mybir as mybir
    import concourse.tile as tile
    from concourse import bacc

    f32 = mybir.dt.float32
    f16 = mybir.dt.float16
    f8 = mybir.dt.float8e4

    nc = bacc.Bacc("TRN2", target_bir_lowering=False, debug=False)

    kT = nc.declare_dram_parameter("kT", [H, LK], f8, isOutput=False)
    qT = nc.declare_dram_parameter("qT", [H, NQ], f8, isOutput=False)
    vA = nc.declare_dram_parameter("vA", [LK, HA], f16, isOutput=False)
    out = nc.declare_dram_parameter("out", [NQ, H], f32, isOutput=True)

    # [h, s] -> [p, ho, s] with h = ho*128 + p
    qT_r = qT.ap().rearrange("(o p) n -> p o n", p=P)
    kT_r = kT.ap().rearrange("(o p) n -> p o n", p=P)
    vA_r = vA.ap().rearrange("(t p) c -> p t c", p=P)

    Exp = mybir.ActivationFunctionType.Exp
    DR = mybir.MatmulPerfMode.DoubleRow

    with tile.TileContext(nc) as tc:
        with tc.tile_pool(name="persist", bufs=1) as persist:
            kraw = persist.tile([P, HO, LK], f8)
            qraw = persist.tile([P, HO, NQ], f8)
            V_sb = persist.tile([P, KT, HA], f16)  # e^u * [v, 1], fp16

            # DMA issue order = consumption order. The first critical
            # loads are issued from four different engines in parallel
            # (each dma_start costs ~0.9us of issue time on its engine),
            # so the first score matmul isn't gated on serialized issues.
            def g(eng, dst, src, lo, hi):
                eng.dma_start(dst[:, :, lo:hi], src[:, :, lo:hi])
            def gv(eng, lo, hi):
                eng.dma_start(V_sb[:, lo:hi, :], vA_r[:, lo:hi, :])
            g(nc.sync, qraw, qT_r, 0, QC)
            g(nc.scalar, kraw, kT_r, 0, 128)
            g(nc.scalar, kraw, kT_r, 128, 512)
            gv(nc.sync, 0, 8)
            g(nc.gpsimd, kraw, kT_r, 512, 1024)
            g(nc.gpsimd, kraw, kT_r, 1024, 2048)
            gv(nc.gpsimd, 8, 16)
            g(nc.gpsimd, qraw, qT_r, QC, 2 * QC)
            g(nc.gpsimd, kraw, kT_r, 2048, 3072)
            gv(nc.gpsimd, 16, 24)
            g(nc.gpsimd, kraw, kT_r, 3072, 4096)
            gv(nc.gpsimd, 24, 32)
            g(nc.gpsimd, qraw, qT_r, 2 * QC, 3 * QC)
            g(nc.gpsimd, qraw, qT_r, 3 * QC, 4 * QC)

            with (
                tc.tile_pool(name="pt", bufs=8) as ptp,
                tc.tile_pool(name="ps_s", bufs=2, space="PSUM") as pss,
                tc.tile_pool(name="ps_ctx", bufs=4, space="PSUM") as psc,
                tc.tile_pool(name="fin", bufs=8) as fin,
            ):
                NPAIR = NQC * KTP  # 64 global pair steps
                pts = {}    # global pair idx -> P^T tile
                ctxs = {}   # chunk idx -> list of 4 ctx psum tiles

                def scores_mm(gp, j):
                    # DoubleRow: lhsT [128, 2, 128] contracts 256 h in
                    # one pass; rhs [128, 2, 512] -> out [128, 512]
                    qc, tp = divmod(gp, KTP)
                    kt = HO * tp + j
                    nc.tensor.matmul(
                        pts[gp][0][:, j, :],
                        kraw[:, :, kt * P:(kt + 1) * P],
                        qraw[:, :, qc * QC:(qc + 1) * QC],
                        start=True,
                        stop=True,
                        perf_mode=DR,
                    )

                def pv_half(gp, qws):
                    # P@V matmuls for lagged pair gp, q-windows qws; when a
                    # q-window sees its chunk's last k-tile, its normalize
                    # and output DMA are fused in immediately.
                    qc, tp = divmod(gp, KTP)
                    pt = pts[gp][1]
                    ctx = ctxs[qc]
                    for qw in qws:
                        for j in range(HO):
                            kt = HO * tp + j
                            nc.tensor.matmul(
                                ctx[qw][:],
                                pt[:, j, qw * P:(qw + 1) * P],
                                V_sb[:, kt, :],
                                start=(kt == 0),
                                stop=(kt == KT - 1),
                            )
                        if tp == KTP - 1:
                            rec = fin.tile([P, 1], f32, tag="rec")
                            nc.vector.reciprocal(rec[:], ctx[qw][:, H:HA])
                            osb = fin.tile([P, H], f32, tag="osb")
                            nc.vector.tensor_scalar_mul(
                                osb[:], ctx[qw][:, :H], rec[:])
                            nc.sync.dma_start(
                                out.ap()[qc * QC + qw * P:
                                         qc * QC + (qw + 1) * P, :],
                                osb[:],
                            )

                # Flattened software pipeline over all 64 pairs: P@V lags
                # scores by LAGP pairs (hides the exp's ScalarE latency)
                # and crosses chunk boundaries, so the PE never drains
                # between chunks. The two DoubleRow scores matmuls are
                # spaced by P@V work so their (serial) 256-col weight
                # loads overlap P@V streaming.
                for gp in range(NPAIR + LAGP):
                    if gp < NPAIR:
                        qc, tp = divmod(gp, KTP)
                        if tp == 0:
                            ctxs[qc] = [
                                psc.tile([P, HA], f32, tag="ps_ctx",
                                         name=f"ctx_{qc}_{qw}")
                                for qw in range(QW)
                            ]
                        ps = pss.tile([P, HO, QC], f32, tag="ps_s")
                        pt = ptp.tile([P, HO, QC], f16, tag="pt")
                        pts[gp] = (ps, pt)
                        scores_mm(gp, 0)
                        if gp >= LAGP:
                            pv_half(gp - LAGP, (0, 1))
                        scores_mm(gp, 1)
                        nc.scalar.activation(pt[:], ps[:], Exp,
                                             scale=float(SCALE))
                        if gp >= LAGP:
                            pv_half(gp - LAGP, (2, 3))
                            del pts[gp - LAGP]
                    else:
                        pv_half(gp - LAGP, (0, 1))
                        pv_half(gp - LAGP, (2, 3))
                        del pts[gp - LAGP]
    nc.compile()
    return nc


def _get_nc():
    global _NC_CACHE
    if _NC_CACHE is None:
        _NC_CACHE = _build_nc()
    return _NC_CACHE


def _prep_in_maps(q, k, v, Wq, bq, Wk, bk, Wv, bv):
    q = np.asarray(q, np.float32)
    k = np.asarray(k, np.float32)
    v = np.asarray(v, np.float32)
    Wq = np.asarray(Wq, np.float64)
    Wk = np.asarray(Wk, np.float64)
    bq_ = np.asarray(bq, np.float64)
    bk_ = np.asarray(bk, np.float64)
    M = Wq.T @ Wk                       # [h, h~]
    w2v = Wk.T @ bq_                    # [h]
    ccv = float(bq_ @ bk_)
    M32 = M.astype(np.float32)
    Wv32 = np.asarray(Wv, np.float32)
    bv32 = np.asarray(bv, np.float32)
    in_maps = []
    for i in range(NCORES):
        b, half = divmod(i, NCORES // B)
        qm = q[b, half * NQ:(half + 1) * NQ, :] @ M32   # fold M: scores = (qM) k^T
        qT_i = np.ascontiguousarray(qm.T).astype(_F8)
        kT_i = np.ascontiguousarray(k[b].T).astype(_F8)
        # u_k = (k.(Wk.T bq) + bq.bk)/sqrt(H); e^{u_k} folded into V rows
        u = (k[b].astype(np.float64) @ w2v + ccv) * float(SCALE)
        eu = np.exp(u)
        vA_i = np.empty((LK, HA), np.float64)
        vA_i[:, :H] = v[b] @ Wv32.T + bv32
        vA_i[:, H] = 1.0
        vA_i *= eu[:, None]
        vA_i = vA_i.astype(_F16)
        in_maps.append({
            "qT": qT_i, "kT": kT_i, "vA": vA_i,
        })
    return in_maps


def _install_ntff_hook_shim():
    """The image's antenv lacks axon_hooks; recreate it from the boot recipe
    (ctypes into libaxon_pjrt.so) so trace=True can capture NTFF profiles."""
    import types
    import contextlib
    import ctypes

    if "antenv.axon_hooks" in sys.modules:
        return
    so_path = "/opt/axon/libaxon_pjrt.so"
    hook = None
    if os.path.exists(so_path):
        lib = ctypes.CDLL(so_path)
        if hasattr(lib, "axon_start_nrt_profile"):
            lib.axon_start_nrt_profile.argtypes = [
                ctypes.POINTER(ctypes.c_int64), ctypes.c_size_t]
            lib.axon_start_nrt_profile.restype = ctypes.c_int64
            lib.axon_stop_nrt_profile.argtypes = [ctypes.c_char_p]
            lib.axon_stop_nrt_profile.restype = ctypes.c_int64

            @contextlib.contextmanager
            def _hook(output_dir, device_ids):
                import jax
                jax.devices()
                if device_ids:
                    ids = (ctypes.c_int64 * len(device_ids))(*device_ids)
                    rc = lib.axon_start_nrt_profile(ids, len(device_ids))
                else:
                    rc = lib.axon_start_nrt_profile(None, 0)
                if rc != 0:
                    raise RuntimeError(f"axon_start_nrt_profile rc={rc}")
                try:
                    yield
                finally:
                    n = lib.axon_stop_nrt_profile(str(output_dir).encode())
                    print(f"profile: {n} file(s) written to {output_dir}")

            hook = _hook
    mod = types.ModuleType("antenv.axon_hooks")
    mod.get_axon_ntff_profile_hook = lambda: hook
    mod.set_axon_ntff_profile_hook = lambda h: None
    sys.modules["antenv.axon_hooks"] = mod


def run(inputs, trace=False, trace_cores=None):
    """Run on 8 NeuronCores. Returns (output, BassKernelResults)."""
    from concourse.bass_utils import run_bass_kernel_spmd

    if trace:
        _install_ntff_hook_shim()
    nc = _get_nc()
    in_maps = _prep_in_maps(**inputs)
    res = run_bass_kernel_spmd(
        nc, in_maps, core_ids=list(range(NCORES)),
        trace=trace, trace_cores=trace_cores,
    )
    full = np.empty((B, LQ, H), np.float32)
    for i in range(NCORES):
        b, half = divmod(i, NCORES // B)
        full[b, half * NQ:(half + 1) * NQ, :] = res.results[i]["out"]
    return full, res


def kernel(**inputs):
    return run(inputs, trace=False)[0]


# revision 8
# speedup vs baseline: 1.0274x; 1.0274x over previous
"""Cross-attention kernel for Trainium2 (8 NeuronCores, SPMD).

Problem: B=4, LQ=LK=4096, H=256
  query = q @ Wq.T + bq ; keys = k @ Wk.T + bk ; values = v @ Wv.T + bv
  out = softmax(query @ keys.T / sqrt(H)) @ values

Sharding: core i -> batch i//2, query rows (i%2)*2048 .. +2048.
K/V for the batch are replicated across the 2 cores sharing it.

Device algorithm (PE contracts over the partition dim):
  - scores are algebraically refactored:
      s[q,k] = (q M)_q k_k^T + t_q + u_k,  M = Wq.T @ Wk
      t_q -- constant per softmax row: cancels, dropped
      u_k = (k·(Wk.T bq) + bq·bk)/sqrt(H) -- per-key scalar; exp(u_k) is
            folded into the V rows on the host (V' = e^u [v, 1]), so the
            device exp needs NO bias and activations can span k-tiles.
  - qM and k are quantized to fp8e4 on the host (RNE); the scores matmul
    runs in DoubleRow perf mode: one matmul per k-tile contracts all 256
    h in a single pass ([128, 2, x] APs pair h and h+128 per partition),
    2x the bf16 rate.
  - scores are computed transposed ([k, q]) so exp(scores) = P^T is born
    k-major; P is materialized fp16 (attention here is diffuse, so fp8
    P/V error would not average down enough -- fp16 keeps P@V exact-ish
    while fp8 scores stay within the error budget).
  - softmax skips max-subtraction (scaled scores stay within ~+-4).
  - exp runs on ScalarE over [128, 1024] k-tile PAIRS (two PSUM banks per
    activation) to amortize the ~352-cycle fixed cost per ACTIVATE.
  - P@V uses P^T windows as stationary and V' ([k, 257] fp16, e^u row
    scale, ones-column) as moving: output column 256 is the softmax
    denominator and the context lands in natural [q, h] layout.
  - score and P@V matmuls are interleaved per k-tile-pair (P@V lags LAGP
    pairs) so the exp's ScalarE latency hides behind P@V work on PE; each
    chunk drains qw-major with the normalize fused per q-window.
"""

import os
import sys

import numpy as np

sys.path.insert(0, "/opt/trn_rl_repo")

import ml_dtypes

B, LQ, LK, H = 4, 4096, 4096, 256
P = 128
HO = H // P            # 2 h-tiles
NCORES = 8
NQ = LQ * B // NCORES  # 2048 q rows per core
QC = 512               # q chunk (scores tile width)
NQC = NQ // QC         # 4
QW = QC // P           # 4 q-windows per chunk
KT = LK // P           # 32 k tiles
KTP = KT // 2          # 16 k-tile pairs
HA = H + 1             # V augmented with ones column
LAGP = 3               # P@V lags scores by this many k-tile pairs
SCALE = 1.0 / np.sqrt(np.float32(H))  # 1/16

_F8 = ml_dtypes.float8_e4m3
_F16 = np.float16

_NC_CACHE = None


def _build_nc():
    """Build the single-core Bass program (same program runs SPMD on 8 cores)."""
    import concourse.bass as bass
    import concourse.mybir as mybir
    import concourse.tile as tile
    from concourse import bacc

    f32 = mybir.dt.float32
    f16 = mybir.dt.float16
    f8 = mybir.dt.float8e4

    nc = bacc.Bacc("TRN2", target_bir_lowering=False, debug=False)

    kT = nc.declare_dram_parameter("kT", [H, LK], f8, isOutput=False)
    qT = nc.declare_dram_parameter("qT", [H, NQ], f8, isOutput=False)
    vA = nc.declare_dram_parameter("vA", [LK, HA], f16, isOutput=False)
    out = nc.declare_dram_parameter("out", [NQ, H], f32, isOutput=True)

    # [h, s] -> [p, ho, s] with h = ho*128 + p
    qT_r = qT.ap().rearrange("(o p) n -> p o n", p=P)
    kT_r = kT.ap().rearrange("(o p) n -> p o n", p=P)
    vA_r = vA.ap().rearrange("(t p) c -> p t c", p=P)

    Exp = mybir.ActivationFunctionType.Exp
    DR = mybir.MatmulPerfMode.DoubleRow

    with tile.TileContext(nc) as tc:
        with tc.tile_pool(name="persist", bufs=1) as persist:
            kraw = persist.tile([P, HO, LK], f8)
            qraw = persist.tile([P, HO, NQ], f8)
            V_sb = persist.tile([P, KT, HA], f16)  # e^u * [v, 1], fp16

            # DMA issue order = consumption order. The first critical
            # loads are issued from four different engines in parallel
            # (each dma_start costs ~0.9us of issue time on its engine),
            # so the first score matmul isn't gated on serialized issues.
            def g(eng, dst, src, lo, hi):
                eng.dma_start(dst[:, :, lo:hi], src[:, :, lo:hi])
            def gv(eng, lo, hi):
                eng.dma_start(V_sb[:, lo:hi, :], vA_r[:, lo:hi, :])
            # The input queue drains ~224 GB/s aggregate but processes
            # dma_starts roughly in issue order, so sizes are staged to
            # track the consumption timeline (~145 GB/s): small k/q
            # fronts first, V slabs interleaved just-in-time.
            g(nc.sync, qraw, qT_r, 0, QC)          # mm #1
            g(nc.scalar, kraw, kT_r, 0, 256)       # pairs 0-1
            g(nc.gpsimd, kraw, kT_r, 256, 1024)    # pairs 2-7
            gv(nc.sync, 0, 8)                      # pv from gp=LAGP
            g(nc.gpsimd, kraw, kT_r, 1024, 2048)
            gv(nc.sync, 8, 16)
            g(nc.gpsimd, qraw, qT_r, QC, 2 * QC)
            g(nc.gpsimd, kraw, kT_r, 2048, 3072)
            gv(nc.sync, 16, 24)
            g(nc.gpsimd, kraw, kT_r, 3072, 4096)
            gv(nc.sync, 24, 32)
            g(nc.gpsimd, qraw, qT_r, 2 * QC, 3 * QC)
            g(nc.gpsimd, qraw, qT_r, 3 * QC, 4 * QC)

            with (
                tc.tile_pool(name="pt", bufs=8) as ptp,
                tc.tile_pool(name="ps_s", bufs=2, space="PSUM") as pss,
                tc.tile_pool(name="ps_ctx", bufs=4, space="PSUM") as psc,
                tc.tile_pool(name="fin", bufs=8) as fin,
            ):
                NPAIR = NQC * KTP  # 64 global pair steps
                pts = {}    # global pair idx -> P^T tile
                ctxs = {}   # chunk idx -> list of 4 ctx psum tiles

                def scores_mm(gp, j):
                    # DoubleRow: lhsT [128, 2, 128] contracts 256 h in
                    # one pass; rhs [128, 2, 512] -> out [128, 512]
                    qc, tp = divmod(gp, KTP)
                    kt = HO * tp + j
                    nc.tensor.matmul(
                        pts[gp][0][:, j, :],
                        kraw[:, :, kt * P:(kt + 1) * P],
                        qraw[:, :, qc * QC:(qc + 1) * QC],
                        start=True,
                        stop=True,
                        perf_mode=DR,
                    )

                def pv_half(gp, qws):
                    # P@V matmuls for lagged pair gp, q-windows qws; when a
                    # q-window sees its chunk's last k-tile, its normalize
                    # and output DMA are fused in immediately.
                    qc, tp = divmod(gp, KTP)
                    pt = pts[gp][1]
                    ctx = ctxs[qc]
                    for qw in qws:
                        for j in range(HO):
                            kt = HO * tp + j
                            nc.tensor.matmul(
                                ctx[qw][:],
                                pt[:, j, qw * P:(qw + 1) * P],
                                V_sb[:, kt, :],
                                start=(kt == 0),
                                stop=(kt == KT - 1),
                            )
                        if tp == KTP - 1:
                            rec = fin.tile([P, 1], f32, tag="rec")
                            nc.vector.reciprocal(rec[:], ctx[qw][:, H:HA])
                            osb = fin.tile([P, H], f32, tag="osb")
                            nc.vector.tensor_scalar_mul(
                                osb[:], ctx[qw][:, :H], rec[:])
                            nc.sync.dma_start(
                                out.ap()[qc * QC + qw * P:
                                         qc * QC + (qw + 1) * P, :],
                                osb[:],
                            )

                # Flattened software pipeline over all 64 pairs: P@V lags
                # scores by LAGP pairs (hides the exp's ScalarE latency)
                # and crosses chunk boundaries, so the PE never drains
                # between chunks. The two DoubleRow scores matmuls are
                # spaced by P@V work so their (serial) 256-col weight
                # loads overlap P@V streaming.
                for gp in range(NPAIR + LAGP):
                    if gp < NPAIR:
                        qc, tp = divmod(gp, KTP)
                        if tp == 0:
                            ctxs[qc] = [
                                psc.tile([P, HA], f32, tag="ps_ctx",
                                         name=f"ctx_{qc}_{qw}")
                                for qw in range(QW)
                            ]
                        ps = pss.tile([P, HO, QC], f32, tag="ps_s")
                        pt = ptp.tile([P, HO, QC], f16, tag="pt")
                        pts[gp] = (ps, pt)
                        scores_mm(gp, 0)
                        scores_mm(gp, 1)
                        nc.scalar.activation(pt[:], ps[:], Exp,
                                             scale=float(SCALE))
                        if gp >= LAGP:
                            pv_half(gp - LAGP, (0, 1))
                            pv_half(gp - LAGP, (2, 3))
                            del pts[gp - LAGP]
                    else:
                        pv_half(gp - LAGP, (0, 1))
                        pv_half(gp - LAGP, (2, 3))
                        del pts[gp - LAGP]
    nc.compile()
    return nc


def _get_nc():
    global _NC_CACHE
    if _NC_CACHE is None:
        _NC_CACHE = _build_nc()
    return _NC_CACHE


def _prep_in_maps(q, k, v, Wq, bq, Wk, bk, Wv, bv):
    q = np.asarray(q, np.float32)
    k = np.asarray(k, np.float32)
    v = np.asarray(v, np.float32)
    Wq = np.asarray(Wq, np.float64)
    Wk = np.asarray(Wk, np.float64)
    bq_ = np.asarray(bq, np.float64)
    bk_ = np.asarray(bk, np.float64)
    M = Wq.T @ Wk                       # [h, h~]
    w2v = Wk.T @ bq_                    # [h]
    ccv = float(bq_ @ bk_)
    M32 = M.astype(np.float32)
    Wv32 = np.asarray(Wv, np.float32)
    bv32 = np.asarray(bv, np.float32)
    in_maps = []
    for i in range(NCORES):
        b, half = divmod(i, NCORES // B)
        qm = q[b, half * NQ:(half + 1) * NQ, :] @ M32   # fold M: scores = (qM) k^T
        qT_i = np.ascontiguousarray(qm.T).astype(_F8)
        kT_i = np.ascontiguousarray(k[b].T).astype(_F8)
        # u_k = (k.(Wk.T bq) + bq.bk)/sqrt(H); e^{u_k} folded into V rows
        u = (k[b].astype(np.float64) @ w2v + ccv) * float(SCALE)
        eu = np.exp(u)
        vA_i = np.empty((LK, HA), np.float64)
        vA_i[:, :H] = v[b] @ Wv32.T + bv32
        vA_i[:, H] = 1.0
        vA_i *= eu[:, None]
        vA_i = vA_i.astype(_F16)
        in_maps.append({
            "qT": qT_i, "kT": kT_i, "vA": vA_i,
        })
    return in_maps


def _install_ntff_hook_shim():
    """The image's antenv lacks axon_hooks; recreate it from the boot recipe
    (ctypes into libaxon_pjrt.so) so trace=True can capture NTFF profiles."""
    import types
    import contextlib
    import ctypes

    if "antenv.axon_hooks" in sys.modules:
        return
    so_path = "/opt/axon/libaxon_pjrt.so"
    hook = None
    if os.path.exists(so_path):
        lib = ctypes.CDLL(so_path)
        if hasattr(lib, "axon_start_nrt_profile"):
            lib.axon_start_nrt_profile.argtypes = [
                ctypes.POINTER(ctypes.c_int64), ctypes.c_size_t]
            lib.axon_start_nrt_profile.restype = ctypes.c_int64
            lib.axon_stop_nrt_profile.argtypes = [ctypes.c_char_p]
            lib.axon_stop_nrt_profile.restype = ctypes.c_int64

            @contextlib.contextmanager
            def _hook(output_dir, device_ids):
                import jax
                jax.devices()
                if device_ids:
                    ids = (ctypes.c_int64 * len(device_ids))(*device_ids)
                    rc = lib.axon_start_nrt_profile(ids, len(device_ids))
                else:
                    rc = lib.axon_start_nrt_profile(None, 0)
                if rc != 0:
                    raise RuntimeError(f"axon_start_nrt_profile rc={rc}")
                try:
                    yield
                finally:
                    n = lib.axon_stop_nrt_profile(str(output_dir).encode())
                    print(f"profile: {n} file(s) written to {output_dir}")

            hook = _hook
    mod = types.ModuleType("antenv.axon_hooks")
    mod.get_axon_ntff_profile_hook = lambda: hook
    mod.set_axon_ntff_profile_hook = lambda h: None
    sys.modules["antenv.axon_hooks"] = mod


def run(inputs, trace=False, trace_cores=None):
    """Run on 8 NeuronCores. Returns (output, BassKernelResults)."""
    from concourse.bass_utils import run_bass_kernel_spmd

    if trace:
        _install_ntff_hook_shim()
    nc = _get_nc()
    in_maps = _prep_in_maps(**inputs)
    res = run_bass_kernel_spmd(
        nc, in_maps, core_ids=list(range(NCORES)),
        trace=trace, trace_cores=trace_cores,
    )
    full = np.empty((B, LQ, H), np.float32)
    for i in range(NCORES):
        b, half = divmod(i, NCORES // B)
        full[b, half * NQ:(half + 1) * NQ, :] = res.results[i]["out"]
    return full, res


def kernel(**inputs):
    return run(inputs, trace=False)[0]


# revision 12
# speedup vs baseline: 1.0402x; 1.0125x over previous
"""Cross-attention kernel for Trainium2 (8 NeuronCores, SPMD).

Problem: B=4, LQ=LK=4096, H=256
  query = q @ Wq.T + bq ; keys = k @ Wk.T + bk ; values = v @ Wv.T + bv
  out = softmax(query @ keys.T / sqrt(H)) @ values

Sharding: core i -> batch i//2, query rows (i%2)*2048 .. +2048.
K/V for the batch are replicated across the 2 cores sharing it.

Device algorithm (PE contracts over the partition dim):
  - scores are algebraically refactored:
      s[q,k] = (q M)_q k_k^T + t_q + u_k,  M = Wq.T @ Wk
      t_q -- constant per softmax row: cancels, dropped
      u_k = (k·(Wk.T bq) + bq·bk)/sqrt(H) -- per-key scalar; exp(u_k) is
            folded into the V rows on the host (V' = e^u [v, 1]), so the
            device exp needs NO bias and activations can span k-tiles.
  - qM and k are quantized to fp8e4 on the host (RNE); the scores matmul
    runs in DoubleRow perf mode: one matmul per k-tile contracts all 256
    h in a single pass ([128, 2, x] APs pair h and h+128 per partition),
    2x the bf16 rate.
  - scores are computed transposed ([k, q]) so exp(scores) = P^T is born
    k-major; P is materialized fp16 (attention here is diffuse, so fp8
    P/V error would not average down enough -- fp16 keeps P@V exact-ish
    while fp8 scores stay within the error budget).
  - softmax skips max-subtraction (scaled scores stay within ~+-4).
  - exp runs on ScalarE over [128, 1024] k-tile PAIRS (two PSUM banks per
    activation) to amortize the ~352-cycle fixed cost per ACTIVATE.
  - P@V uses P^T windows as stationary and V' ([k, 257] fp16, e^u row
    scale, ones-column) as moving: output column 256 is the softmax
    denominator and the context lands in natural [q, h] layout.
  - score and P@V matmuls are interleaved per k-tile-pair (P@V lags LAGP
    pairs) so the exp's ScalarE latency hides behind P@V work on PE; each
    chunk drains qw-major with the normalize fused per q-window.
"""

import os
import sys

import numpy as np

sys.path.insert(0, "/opt/trn_rl_repo")

import ml_dtypes

B, LQ, LK, H = 4, 4096, 4096, 256
P = 128
HO = H // P            # 2 h-tiles
NCORES = 8
NQ = LQ * B // NCORES  # 2048 q rows per core
QC = 512               # q chunk (scores tile width)
NQC = NQ // QC         # 4
QW = QC // P           # 4 q-windows per chunk
KT = LK // P           # 32 k tiles
KTP = KT // 2          # 16 k-tile pairs
HA = H + 1             # V augmented with ones column
LAGP = 2               # P@V lags scores by this many k-tile pairs
SCALE = 1.0 / np.sqrt(np.float32(H))  # 1/16

_F8 = ml_dtypes.float8_e4m3
_F16 = np.float16

_NC_CACHE = None


def _build_nc():
    """Build the single-core Bass program (same program runs SPMD on 8 cores)."""
    import concourse.bass as bass
    import concourse.mybir as mybir
    import concourse.tile as tile
    from concourse import bacc

    f32 = mybir.dt.float32
    f16 = mybir.dt.float16
    f8 = mybir.dt.float8e4

    nc = bacc.Bacc("TRN2", target_bir_lowering=False, debug=False)

    kT = nc.declare_dram_parameter("kT", [H, LK], f8, isOutput=False)
    qT = nc.declare_dram_parameter("qT", [H, NQ], f8, isOutput=False)
    vA = nc.declare_dram_parameter("vA", [LK, HA], f16, isOutput=False)
    out = nc.declare_dram_parameter("out", [NQ, H], f32, isOutput=True)

    # [h, s] -> [p, ho, s] with h = ho*128 + p
    qT_r = qT.ap().rearrange("(o p) n -> p o n", p=P)
    kT_r = kT.ap().rearrange("(o p) n -> p o n", p=P)
    vA_r = vA.ap().rearrange("(t p) c -> p t c", p=P)

    Exp = mybir.ActivationFunctionType.Exp
    DR = mybir.MatmulPerfMode.DoubleRow

    with tile.TileContext(nc) as tc:
        with tc.tile_pool(name="persist", bufs=1) as persist:
            kraw = persist.tile([P, HO, LK], f8)
            qraw = persist.tile([P, HO, NQ], f8)
            V_sb = persist.tile([P, KT, HA], f16)  # e^u * [v, 1], fp16

            # DMA issue order = consumption order. The first critical
            # loads are issued from four different engines in parallel
            # (each dma_start costs ~0.9us of issue time on its engine),
            # so the first score matmul isn't gated on serialized issues.
            def g(eng, dst, src, lo, hi):
                eng.dma_start(dst[:, :, lo:hi], src[:, :, lo:hi])
            def gv(eng, lo, hi):
                eng.dma_start(V_sb[:, lo:hi, :], vA_r[:, lo:hi, :])
            # The input queue drains ~224 GB/s aggregate but round-robins
            # packets across every enqueued dma_start, so the critical
            # first transfers (qraw chunk 0, kraw front) are issued alone
            # from sync/scalar while gpsimd's bulk enqueue is held back
            # behind a memset delay; later slabs are staged in
            # consumption order (~145 GB/s drain rate).
            g(nc.sync, qraw, qT_r, 0, QC)          # mm #1
            g(nc.scalar, kraw, kT_r, 0, 256)       # pairs 0-1
            g(nc.scalar, kraw, kT_r, 256, 512)     # pairs 2-3
            delay = persist.tile([P, 1024], mybir.dt.float32)
            nc.gpsimd.memset(delay[:], 0.0)        # hold bulk enqueue back
            gv(nc.gpsimd, 0, 8)                    # pv from gp=LAGP
            g(nc.gpsimd, kraw, kT_r, 512, 1024)
            gv(nc.gpsimd, 8, 16)
            g(nc.gpsimd, kraw, kT_r, 1024, 2048)
            g(nc.gpsimd, qraw, qT_r, QC, 2 * QC)
            gv(nc.gpsimd, 16, 24)
            g(nc.gpsimd, kraw, kT_r, 2048, 3072)
            gv(nc.gpsimd, 24, 32)
            g(nc.gpsimd, kraw, kT_r, 3072, 4096)
            g(nc.gpsimd, qraw, qT_r, 2 * QC, 3 * QC)
            g(nc.gpsimd, qraw, qT_r, 3 * QC, 4 * QC)

            with (
                tc.tile_pool(name="pt", bufs=8) as ptp,
                tc.tile_pool(name="ps_s", bufs=2, space="PSUM") as pss,
                tc.tile_pool(name="ps_ctx", bufs=4, space="PSUM") as psc,
                tc.tile_pool(name="fin", bufs=8) as fin,
            ):
                NPAIR = NQC * KTP  # 64 global pair steps
                pts = {}    # global pair idx -> P^T tile
                ctxs = {}   # chunk idx -> list of 4 ctx psum tiles

                def scores_mm(gp, j):
                    # DoubleRow: lhsT [128, 2, 128] contracts 256 h in
                    # one pass; rhs [128, 2, 512] -> out [128, 512]
                    qc, tp = divmod(gp, KTP)
                    kt = HO * tp + j
                    nc.tensor.matmul(
                        pts[gp][0][:, j, :],
                        kraw[:, :, kt * P:(kt + 1) * P],
                        qraw[:, :, qc * QC:(qc + 1) * QC],
                        start=True,
                        stop=True,
                        perf_mode=DR,
                    )

                def pv_half(gp, qws):
                    # P@V matmuls for lagged pair gp, q-windows qws; when a
                    # q-window sees its chunk's last k-tile, its normalize
                    # and output DMA are fused in immediately.
                    qc, tp = divmod(gp, KTP)
                    pt = pts[gp][1]
                    ctx = ctxs[qc]
                    for qw in qws:
                        for j in range(HO):
                            kt = HO * tp + j
                            nc.tensor.matmul(
                                ctx[qw][:],
                                pt[:, j, qw * P:(qw + 1) * P],
                                V_sb[:, kt, :],
                                start=(kt == 0),
                                stop=(kt == KT - 1),
                            )
                        if tp == KTP - 1:
                            rec = fin.tile([P, 1], f32, tag="rec")
                            nc.vector.reciprocal(rec[:], ctx[qw][:, H:HA])
                            osb = fin.tile([P, H], f32, tag="osb")
                            if qc == NQC - 1:
                                # ScalarE is idle in the tail: offload the
                                # last chunk's normalize so the qw chains
                                # don't serialize on Vector.
                                nc.scalar.mul(osb[:], ctx[qw][:, :H], rec[:])
                            else:
                                nc.vector.tensor_scalar_mul(
                                    osb[:], ctx[qw][:, :H], rec[:])
                            nc.sync.dma_start(
                                out.ap()[qc * QC + qw * P:
                                         qc * QC + (qw + 1) * P, :],
                                osb[:],
                            )

                # Flattened software pipeline over all 64 pairs: P@V lags
                # scores by LAGP pairs (hides the exp's ScalarE latency)
                # and crosses chunk boundaries, so the PE never drains
                # between chunks. The two DoubleRow scores matmuls are
                # spaced by P@V work so their (serial) 256-col weight
                # loads overlap P@V streaming.
                for gp in range(NPAIR + LAGP):
                    if gp < NPAIR:
                        qc, tp = divmod(gp, KTP)
                        if tp == 0:
                            ctxs[qc] = [
                                psc.tile([P, HA], f32, tag="ps_ctx",
                                         name=f"ctx_{qc}_{qw}")
                                for qw in range(QW)
                            ]
                        ps = pss.tile([P, HO, QC], f32, tag="ps_s")
                        pt = ptp.tile([P, HO, QC], f16, tag="pt")
                        pts[gp] = (ps, pt)
                        scores_mm(gp, 0)
                        scores_mm(gp, 1)
                        nc.scalar.activation(pt[:], ps[:], Exp,
                                             scale=float(SCALE))
                        if gp >= LAGP:
                            pv_half(gp - LAGP, (0, 1))
                            pv_half(gp - LAGP, (2, 3))
                            del pts[gp - LAGP]
                    else:
                        pv_half(gp - LAGP, (0, 1))
                        pv_half(gp - LAGP, (2, 3))
                        del pts[gp - LAGP]
    nc.compile()
    return nc


def _get_nc():
    global _NC_CACHE
    if _NC_CACHE is None:
        _NC_CACHE = _build_nc()
    return _NC_CACHE


def _prep_in_maps(q, k, v, Wq, bq, Wk, bk, Wv, bv):
    q = np.asarray(q, np.float32)
    k = np.asarray(k, np.float32)
    v = np.asarray(v, np.float32)
    Wq = np.asarray(Wq, np.float64)
    Wk = np.asarray(Wk, np.float64)
    bq_ = np.asarray(bq, np.float64)
    bk_ = np.asarray(bk, np.float64)
    M = Wq.T @ Wk                       # [h, h~]
    w2v = Wk.T @ bq_                    # [h]
    ccv = float(bq_ @ bk_)
    M32 = M.astype(np.float32)
    Wv32 = np.asarray(Wv, np.float32)
    bv32 = np.asarray(bv, np.float32)
    in_maps = []
    for i in range(NCORES):
        b, half = divmod(i, NCORES // B)
        qm = q[b, half * NQ:(half + 1) * NQ, :] @ M32   # fold M: scores = (qM) k^T
        qT_i = np.ascontiguousarray(qm.T).astype(_F8)
        kT_i = np.ascontiguousarray(k[b].T).astype(_F8)
        # u_k = (k.(Wk.T bq) + bq.bk)/sqrt(H); e^{u_k} folded into V rows
        u = (k[b].astype(np.float64) @ w2v + ccv) * float(SCALE)
        eu = np.exp(u)
        vA_i = np.empty((LK, HA), np.float64)
        vA_i[:, :H] = v[b] @ Wv32.T + bv32
        vA_i[:, H] = 1.0
        vA_i *= eu[:, None]
        vA_i = vA_i.astype(_F16)
        in_maps.append({
            "qT": qT_i, "kT": kT_i, "vA": vA_i,
        })
    return in_maps


def _install_ntff_hook_shim():
    """The image's antenv lacks axon_hooks; recreate it from the boot recipe
    (ctypes into libaxon_pjrt.so) so trace=True can capture NTFF profiles."""
    import types
    import contextlib
    import ctypes

    if "antenv.axon_hooks" in sys.modules:
        return
    so_path = "/opt/axon/libaxon_pjrt.so"
    hook = None
    if os.path.exists(so_path):
        lib = ctypes.CDLL(so_path)
        if hasattr(lib, "axon_start_nrt_profile"):
            lib.axon_start_nrt_profile.argtypes = [
                ctypes.POINTER(ctypes.c_int64), ctypes.c_size_t]
            lib.axon_start_nrt_profile.restype = ctypes.c_int64
            lib.axon_stop_nrt_profile.argtypes = [ctypes.c_char_p]
            lib.axon_stop_nrt_profile.restype = ctypes.c_int64

            @contextlib.contextmanager
            def _hook(output_dir, device_ids):
                import jax
                jax.devices()
                if device_ids:
                    ids = (ctypes.c_int64 * len(device_ids))(*device_ids)
                    rc = lib.axon_start_nrt_profile(ids, len(device_ids))
                else:
                    rc = lib.axon_start_nrt_profile(None, 0)
                if rc != 0:
                    raise RuntimeError(f"axon_start_nrt_profile rc={rc}")
                try:
                    yield
                finally:
                    n = lib.axon_stop_nrt_profile(str(output_dir).encode())
                    print(f"profile: {n} file(s) written to {output_dir}")

            hook = _hook
    mod = types.ModuleType("antenv.axon_hooks")
    mod.get_axon_ntff_profile_hook = lambda: hook
    mod.set_axon_ntff_profile_hook = lambda h: None
    sys.modules["antenv.axon_hooks"] = mod


def run(inputs, trace=False, trace_cores=None):
    """Run on 8 NeuronCores. Returns (output, BassKernelResults)."""
    from concourse.bass_utils import run_bass_kernel_spmd

    if trace:
        _install_ntff_hook_shim()
    nc = _get_nc()
    in_maps = _prep_in_maps(**inputs)
    res = run_bass_kernel_spmd(
        nc, in_maps, core_ids=list(range(NCORES)),
        trace=trace, trace_cores=trace_cores,
    )
    full = np.empty((B, LQ, H), np.float32)
    for i in range(NCORES):
        b, half = divmod(i, NCORES // B)
        full[b, half * NQ:(half + 1) * NQ, :] = res.results[i]["out"]
    return full, res


def kernel(**inputs):
    return run(inputs, trace=False)[0]
